# revision 2
# baseline (speedup 1.0000x reference)
"""DSGIAT GraphBranch kernel for trn2 (8 NeuronCores) — single device call.

Sharding: 4 attention heads (128-wide feature slices) x 2 node halves.
The whole network after h1_pre = x@W1 runs on device in ONE program:
  conv1 (attention coeffs computed on device, exp without max-subtraction),
  LP x2, h2_pre = h1_lp @ W2 (quad AllGather of transposed LP output),
  conv2 (device attention), LP x2, per-graph pooling.
Host: x@W1 GEMM, es1/ed1 fold, token packing, x pooling, final MLP.
"""
import os
import time
import numpy as np
import ml_dtypes
from contextlib import ExitStack

try:
    import jax
    jax.config.update("jax_compilation_cache_dir", "/root/.jax_bass_cache")
    jax.config.update("jax_persistent_cache_min_entry_size_bytes", -1)
    jax.config.update("jax_persistent_cache_min_compile_time_secs", 0.0)
except Exception:
    pass

BF = ml_dtypes.bfloat16

N = 30000
IN_CH = 256
HID = 128
HEADS = 4
OUT1 = 512
NG = 64
ALPHA = 0.5
NEG = 0.2
NCORES = 8
HBLK = 118                  # node blocks per half
HN = HBLK * 128             # 15104
NPAD = 2 * HN               # 30208
PAD_ROW = NPAD - 1
WX = 256                    # extended row width: [h(128) | es | 1 | 0pad]

_cached = {}


def _blob_layout(EBl):
    """Row layout (rows of 128 bf16) of the single packed input blob.
    Shared between host packing and device unpacking."""
    TL = HBLK * EBl
    trows = TL // 128               # rows per table
    tpad = ((3 * trows + 3) // 4) * 4   # padded tables region
    tch = tpad // 4                 # per-core chunk rows
    pieces = [
        ("xt", (IN_CH // 4) * HN // 128),
        ("tchunk", tch),
        ("batv", HBLK),
        ("bias1", 128 * HID * 2 // 128),
        ("bias2", 128 * HID * 2 // 128),
        ("iota", 128),
        ("ident", 128),
        ("w1", IN_CH),
        ("w2", OUT1),
        ("a1v", 2),
        ("a2v", 2),
    ]
    lay = {}
    r = 0
    for k, nr in pieces:
        lay[k] = (r, nr)
        r += nr
    lay["total"] = r
    lay["trows"] = trows
    lay["tpad"] = tpad
    lay["tch"] = tch
    return lay


# ---------------------------------------------------------------- device ---

def _build_program(EBl, debug=False):
    import concourse.tile as tile
    from concourse import bacc, mybir, library_config

    f32, bf, i16 = mybir.dt.float32, mybir.dt.bfloat16, mybir.dt.int16
    AOP = mybir.AluOpType
    ACT = mybir.ActivationFunctionType
    TL = HBLK * EBl
    CL = EBl // 128

    nc = bacc.Bacc("TRN2", target_bir_lowering=False, debug=False,
                   num_devices=NCORES)
    LAY = _blob_layout(EBl)
    blob = nc.dram_tensor("blob", [LAY["total"], 128], bf,
                          kind="ExternalInput")
    tchv = nc.dram_tensor("tchv", [LAY["tch"], 128], bf)
    tfull = nc.dram_tensor("tfull", [LAY["tpad"], 128], bf)
    srcs = nc.dram_tensor("srcs", [16, TL // 16], i16)
    dstf = nc.dram_tensor("dstf", [128, TL // 128], bf)
    wl = nc.dram_tensor("wl", [128, TL // 128], bf)
    batv = nc.dram_tensor("batv", [128, HBLK], bf)

    pool_out = nc.dram_tensor("pool_out", [2 * NG, HID], f32,
                              kind="ExternalOutput")

    xtc = nc.dram_tensor("xtc", [IN_CH // 4, HN], bf)
    xtf = nc.dram_tensor("xtf", [IN_CH, HN], bf)
    hx1h = nc.dram_tensor("hx1h", [HN, WX], bf)
    hx1f = nc.dram_tensor("hx1f", [NPAD, WX], bf)
    aux1d = nc.dram_tensor("aux1d", [HN, 2], f32)
    y0h = nc.dram_tensor("y0h", [HN, HID], bf)
    y0f = nc.dram_tensor("y0f", [NPAD, HID], bf)
    y1h = nc.dram_tensor("y1h", [HN, HID], bf)
    y1f = nc.dram_tensor("y1f", [NPAD, HID], bf)
    y1t = nc.dram_tensor("y1t", [HID, HN], bf)
    y1tf = nc.dram_tensor("y1tf", [OUT1, HN], bf)
    hxh = nc.dram_tensor("hxh", [HN, WX], bf)
    hxf = nc.dram_tensor("hxf", [NPAD, WX], bf)
    aux2 = nc.dram_tensor("aux2", [HN, 2], f32)
    y2h = nc.dram_tensor("y2h", [HN, HID], bf)
    y2f = nc.dram_tensor("y2f", [NPAD, HID], bf)
    y3h = nc.dram_tensor("y3h", [HN, HID], bf)
    y3f = nc.dram_tensor("y3f", [NPAD, HID], bf)
    dbg_t = {}
    if debug:
        for nm, t in (("y0h", y0h), ("y1h", y1h), ("y1t", y1t),
                      ("hxh", hxh), ("aux2", aux2), ("y2h", y2h),
                      ("y3h", y3h)):
            dbg_t[nm] = nc.dram_tensor(
                "dbg_" + nm, list(t.shape),
                f32 if nm == "aux2" else bf, kind="ExternalOutput")
    pairs = [[0, 1], [2, 3], [4, 5], [6, 7]]
    quads = [[0, 2, 4, 6], [1, 3, 5, 7]]

    GMAX = 1024
    HW_LOOPS = os.environ.get("K_PYLOOPS", "") == ""
    UNROLL = int(os.environ.get("K_UNROLL", "4"))

    with tile.TileContext(nc) as tc, ExitStack() as ctx:
        const = ctx.enter_context(tc.tile_pool(name="const", bufs=1))

        nc.gpsimd.load_library(library_config.mlp)

        from concourse.bass import ts

        def brows(key):
            r0, nr = LAY[key]
            return blob[r0:r0 + nr, :]

        def ld_blob(name, key, shape, dt):
            s = const.tile(shape, dt, name=name)
            ap = brows(key)
            if dt == f32:
                ap = ap.bitcast(f32)
            nc.sync.dma_start(s[:], ap)
            return s

        # tables ship quarter-sharded; quad-AllGather reassembles, then
        # unpack to per-table internal DRAM (flat copies)
        TR = LAY["trows"]
        nc.sync.dma_start(tchv[:, :], brows("tchunk"))
        nc.gpsimd.collective_compute(
            "AllGather", mybir.AluOpType.bypass,
            replica_groups=quads, ins=[tchv[:]], outs=[tfull[:]])
        nc.sync.dma_start(srcs[:, :], tfull[0:TR, :].bitcast(i16))
        nc.sync.dma_start(dstf[:, :], tfull[TR:2 * TR, :])
        nc.sync.dma_start(wl[:, :], tfull[2 * TR:3 * TR, :])
        nc.sync.dma_start(batv[:, :], brows("batv"))

        iota_sb = ld_blob("iota_sb", "iota", [128, 128], bf)
        ident_sb = ld_blob("ident_sb", "ident", [128, 128], bf)
        bias1_sb = ld_blob("bias1_sb", "bias1", [128, HID], f32)
        bias2_sb = ld_blob("bias2_sb", "bias2", [128, HID], f32)
        batv_sb = ld_blob("batv_sb", "batv", [128, HBLK], bf)
        a2v_sb = ld_blob("a2v_sb", "a2v", [128, 2], bf)
        a1v_sb = ld_blob("a1v_sb", "a1v", [128, 2], bf)
        w2_sb = [None] * HEADS
        for g in range(HEADS):
            r0, nr = LAY["w2"]
            w2_sb[g] = const.tile([128, HID], bf, name=f"w2_{g}")
            nc.sync.dma_start(w2_sb[g][:],
                              blob[r0 + g * 128:r0 + (g + 1) * 128, :])
        w1_sb = [None] * 2
        for g in range(2):
            r0, nr = LAY["w1"]
            w1_sb[g] = const.tile([128, HID], bf, name=f"w1_{g}")
            nc.sync.dma_start(w1_sb[g][:],
                              blob[r0 + g * 128:r0 + (g + 1) * 128, :])
        ones_sb = const.tile([128, 128], bf, name="ones_sb")
        nc.vector.memset(ones_sb[:], 1.0)
        pool1 = const.tile([NG, HID], f32, name="pool1")
        pool2 = const.tile([NG, HID], f32, name="pool2")
        nc.gpsimd.memset(pool1[:], 0.0)
        nc.gpsimd.memset(pool2[:], 0.0)

        EB16 = EBl // 16

        def for_blocks(body):
            if HW_LOOPS:
                tc.For_i_unrolled(0, HBLK, 1, body, max_unroll=UNROLL)
            else:
                for b in range(HBLK):
                    body(b)

        def load_tok(rp, i, need_wl):
            idx = rp.tile([128, EB16], i16, tag="idx", name="idx")
            for k in range(8):
                nc.sync.dma_start(idx[16 * k:16 * (k + 1), :],
                                  srcs[:, ts(i, EB16)])
            dfb = rp.tile([128, CL], bf, tag="dfb", name="dfb")
            nc.sync.dma_start(dfb[:], dstf[:, ts(i, CL)])
            wlb = None
            if need_wl:
                wlb = rp.tile([128, CL], bf, tag="wlb", name="wlb")
                nc.sync.dma_start(wlb[:], wl[:, ts(i, CL)])
            return idx, dfb, wlb

        def gather(mp_pool, src_t, width, idx, tag):
            msg = mp_pool.tile([128, CL, width], bf, tag=tag, name=tag)
            for g0 in range(0, EBl, GMAX):
                gn = min(GMAX, EBl - g0)
                nc.gpsimd.dma_gather(
                    msg[:, g0 // 128:(g0 + gn) // 128, :], src_t[:, :],
                    idx[:, g0 // 16:(g0 + gn) // 16],
                    gn, gn, width)
            return msg

        def mk_onehot(sp, dfb):
            onehot = sp.tile([128, CL, 128], bf, tag="oh", name="oh")
            nc.vector.tensor_tensor(
                out=onehot[:],
                in0=dfb[:, :, None].to_broadcast([128, CL, 128]),
                in1=iota_sb[:, None, :].to_broadcast([128, CL, 128]),
                op=AOP.is_equal)
            return onehot

        def conv_pass(src_full, hsrc, aux, bias_sb, out_dram):
            """GAT conv: on-device softmax (no max-sub) + self loop + bias,
            relu -> out_dram."""
            with tc.tile_pool(name="cmp", bufs=3) as mp, \
                 tc.tile_pool(name="csp", bufs=3) as sp, \
                 tc.tile_pool(name="crp", bufs=4) as rp, \
                 tc.tile_pool(name="cpp", bufs=2, space="PSUM") as pp, \
                 tc.tile_pool(name="cpe", bufs=2, space="PSUM") as pe:
                def body(b):
                    idx, dfb, _ = load_tok(rp, b, False)
                    msg = gather(mp, src_full, WX, idx, "cmsg")
                    onehot = mk_onehot(sp, dfb)
                    aux_b = rp.tile([128, 2], f32, tag="aux", name="aux_b")
                    nc.sync.dma_start(aux_b[:], aux[ts(b, 128), :])
                    diag = rp.tile([128, 128], bf, tag="diag", name="diag")
                    nc.vector.tensor_scalar(
                        out=diag[:], in0=ident_sb[:], scalar1=aux_b[:, 0:1],
                        scalar2=None, op0=AOP.mult)
                    edi_p = pe.tile([128, 128], f32, space="PSUM", tag="edip",
                                    name="edi_p")
                    nc.tensor.matmul(edi_p[:], lhsT=ones_sb[:], rhs=diag[:],
                                     start=True, stop=True)
                    edi = rp.tile([128, 128], bf, tag="edi", name="edi")
                    nc.vector.tensor_copy(edi[:], edi_p[:])
                    lg = sp.tile([128, CL, 128], bf, tag="lg", name="lg")
                    nc.vector.tensor_tensor(
                        out=lg[:],
                        in0=msg[:, :, 128:129].to_broadcast([128, CL, 128]),
                        in1=edi[:, None, :].to_broadcast([128, CL, 128]),
                        op=AOP.add)
                    el = sp.tile([128, CL, 128], bf, tag="el", name="el")
                    nc.vector.scalar_tensor_tensor(
                        out=el[:], in0=lg[:], scalar=NEG, in1=lg[:],
                        op0=AOP.mult, op1=AOP.max)
                    ex = sp.tile([128, CL, 128], bf, tag="ex", name="ex")
                    nc.scalar.activation(ex[:], el[:], ACT.Exp)
                    selw = sp.tile([128, CL, 128], bf, tag="selw", name="selw")
                    nc.vector.tensor_tensor(out=selw[:], in0=onehot[:],
                                            in1=ex[:], op=AOP.mult)
                    acc = pp.tile([128, 130], f32, space="PSUM", tag="acc",
                                  name="acc")
                    for c in range(CL):
                        nc.tensor.matmul(acc[:], lhsT=selw[:, c, :],
                                         rhs=msg[:, c, 0:130],
                                         start=(c == 0), stop=(c == CL - 1))
                    hd = rp.tile([128, HID], bf, tag="hd", name="hd")
                    nc.sync.dma_start(hd[:], hsrc[ts(b, 128), 0:HID])
                    hd32 = rp.tile([128, HID], f32, tag="hd32", name="hd32")
                    nc.vector.tensor_copy(hd32[:], hd[:])
                    num = rp.tile([128, HID], f32, tag="num", name="num")
                    nc.vector.scalar_tensor_tensor(
                        out=num[:], in0=hd32[:], scalar=aux_b[:, 1:2],
                        in1=acc[:, 0:HID], op0=AOP.mult, op1=AOP.add)
                    den = rp.tile([128, 1], f32, tag="den", name="den")
                    nc.vector.tensor_tensor(out=den[:], in0=acc[:, 129:130],
                                            in1=aux_b[:, 1:2], op=AOP.add)
                    denr = rp.tile([128, 1], f32, tag="denr", name="denr")
                    nc.vector.reciprocal_approx_fast(denr[:], den[:])
                    res = rp.tile([128, HID], f32, tag="res", name="res")
                    nc.vector.scalar_tensor_tensor(
                        out=res[:], in0=num[:], scalar=denr[:, 0:1],
                        in1=bias_sb[:], op0=AOP.mult, op1=AOP.add)
                    r = rp.tile([128, HID], bf, tag="r", name="r")
                    nc.vector.tensor_scalar_max(out=r[:], in0=res[:],
                                                scalar1=0.0)
                    nc.sync.dma_start(out_dram[ts(b, 128), :], r[:])
                for_blocks(body)

        def lp_pass(src_full, res_dram, out_dram, pool_acc, transpose_to):
            with tc.tile_pool(name="lmp", bufs=3) as mp, \
                 tc.tile_pool(name="lsp", bufs=3) as sp, \
                 tc.tile_pool(name="lrp", bufs=4) as rp, \
                 tc.tile_pool(name="lpp", bufs=2, space="PSUM") as pp, \
                 tc.tile_pool(name="lpq", bufs=2, space="PSUM") as pq:
                def body(b):
                    idx, dfb, wlb = load_tok(rp, b, True)
                    msg = gather(mp, src_full, HID, idx, "lmsg")
                    onehot = mk_onehot(sp, dfb)
                    selw = sp.tile([128, CL, 128], bf, tag="selw", name="selw")
                    nc.vector.tensor_tensor(
                        out=selw[:], in0=onehot[:],
                        in1=wlb[:, :, None].to_broadcast([128, CL, 128]),
                        op=AOP.mult)
                    acc = pp.tile([128, HID], f32, space="PSUM", tag="acc",
                                  name="acc")
                    for c in range(CL):
                        nc.tensor.matmul(acc[:], lhsT=selw[:, c, :],
                                         rhs=msg[:, c, :],
                                         start=(c == 0), stop=(c == CL - 1))
                    rt = rp.tile([128, HID], bf, tag="rt", name="rt")
                    nc.sync.dma_start(rt[:], res_dram[ts(b, 128), :])
                    rt32 = rp.tile([128, HID], f32, tag="rt32", name="rt32")
                    nc.vector.tensor_copy(rt32[:], rt[:])
                    t = rp.tile([128, HID], f32, tag="t", name="t")
                    nc.vector.scalar_tensor_tensor(
                        out=t[:], in0=rt32[:], scalar=0.5, in1=acc[:],
                        op0=AOP.mult, op1=AOP.add)
                    r = rp.tile([128, HID], bf, tag="r", name="r")
                    nc.vector.tensor_scalar(out=r[:], in0=t[:], scalar1=1.0,
                                            scalar2=0.0, op0=AOP.min,
                                            op1=AOP.max)
                    if out_dram is not None:
                        nc.sync.dma_start(out_dram[ts(b, 128), :], r[:])
                    if pool_acc is not None:
                        bv = rp.tile([128, 1], bf, tag="bv", name="bv")
                        nc.sync.dma_start(bv[:], batv[:, ts(b, 1)])
                        selb = rp.tile([128, NG], bf, tag="selb", name="selb")
                        nc.vector.tensor_tensor(
                            out=selb[:],
                            in0=bv[:, 0:1].to_broadcast([128, NG]),
                            in1=iota_sb[:, 0:NG], op=AOP.is_equal)
                        pacc = pq.tile([NG, HID], f32, space="PSUM",
                                       tag="pacc", name="pacc")
                        nc.tensor.matmul(pacc[:], lhsT=selb[:], rhs=r[:],
                                         start=True, stop=True)
                        nc.vector.tensor_tensor(out=pool_acc[:],
                                                in0=pool_acc[:], in1=pacc[:],
                                                op=AOP.add)
                    if transpose_to is not None:
                        pt = pq.tile([128, HID], bf, space="PSUM", tag="pt",
                                     name="pt")
                        nc.tensor.transpose(pt[:], r[:], ident_sb[:])
                        rT = rp.tile([128, HID], bf, tag="rT", name="rT")
                        nc.vector.tensor_copy(rT[:], pt[:])
                        nc.sync.dma_start(
                            transpose_to[:, ts(b, 128)], rT[:])
                for_blocks(body)

        def h_pass(srcT, KC, w_sb, av_sb, hx_out, aux_out):
            """h = y @ W from K-chunk transposed srcT [KC*128, HN]; builds
            hx_out rows [h | es | 1 | 0] and aux_out [ed, a_self]."""
            with tc.tile_pool(name="hrp", bufs=4) as rp, \
                 tc.tile_pool(name="hpp", bufs=2, space="PSUM") as pp, \
                 tc.tile_pool(name="hpq", bufs=2, space="PSUM") as pq, \
                 tc.tile_pool(name="hpe", bufs=2, space="PSUM") as pe:
                def body(b):
                    yt = [None] * KC
                    for g in range(KC):
                        yt[g] = rp.tile([128, 128], bf, tag=f"yt{g}",
                                        name=f"yt{g}")
                        nc.sync.dma_start(
                            yt[g][:],
                            srcT[g * 128:(g + 1) * 128, ts(b, 128)])
                    h2p = pp.tile([128, HID], f32, space="PSUM", tag="h2p",
                                  name="h2p")
                    h2tp = pq.tile([128, HID], f32, space="PSUM", tag="h2tp",
                                   name="h2tp")
                    for g in range(KC):
                        nc.tensor.matmul(h2p[:], lhsT=yt[g][:],
                                         rhs=w_sb[g][:],
                                         start=(g == 0), stop=(g == KC - 1))
                    for g in range(KC):
                        nc.tensor.matmul(h2tp[:], lhsT=w_sb[g][:],
                                         rhs=yt[g][:],
                                         start=(g == 0), stop=(g == KC - 1))
                    h2t = rp.tile([128, HID], bf, tag="h2t", name="h2t")
                    nc.vector.tensor_copy(h2t[:], h2tp[:])
                    pes = pe.tile([128, 2], f32, space="PSUM", tag="pes",
                                  name="pes")
                    nc.tensor.matmul(pes[:], lhsT=h2t[:], rhs=av_sb[:],
                                     start=True, stop=True)
                    pes_sb = rp.tile([128, 2], f32, tag="pessb",
                                     name="pes_sb")
                    nc.vector.tensor_copy(pes_sb[:], pes[:])
                    t1 = rp.tile([128, 1], f32, tag="t1", name="t1")
                    nc.vector.tensor_tensor(out=t1[:], in0=pes_sb[:, 0:1],
                                            in1=pes_sb[:, 1:2], op=AOP.add)
                    t2 = rp.tile([128, 1], f32, tag="t2", name="t2")
                    nc.vector.scalar_tensor_tensor(
                        out=t2[:], in0=t1[:], scalar=NEG, in1=t1[:],
                        op0=AOP.mult, op1=AOP.max)
                    t3 = rp.tile([128, 1], f32, tag="t3", name="t3")
                    nc.scalar.activation(t3[:], t2[:], ACT.Exp)
                    aux_t = rp.tile([128, 2], f32, tag="auxt", name="aux_t")
                    nc.vector.tensor_copy(aux_t[:, 0:1], pes_sb[:, 1:2])
                    nc.vector.tensor_copy(aux_t[:, 1:2], t3[:])
                    nc.sync.dma_start(aux_out[ts(b, 128), :], aux_t[:])
                    hxt = rp.tile([128, WX], bf, tag="hxt", name="hxt")
                    nc.vector.tensor_copy(hxt[:, 0:HID], h2p[:])
                    nc.vector.tensor_copy(hxt[:, HID:HID + 1], pes_sb[:, 0:1])
                    nc.vector.memset(hxt[:, HID + 1:HID + 2], 1.0)
                    nc.vector.memset(hxt[:, HID + 2:WX], 0.0)
                    nc.sync.dma_start(hx_out[ts(b, 128), :], hxt[:])
                for_blocks(body)

        def ag(kind_groups, src, dst_t):
            nc.gpsimd.collective_compute(
                "AllGather", mybir.AluOpType.bypass,
                replica_groups=kind_groups, ins=[src[:]], outs=[dst_t[:]])

        STAGES = int(os.environ.get("K_STAGES", "6"))
        nc.sync.dma_start(xtc[:, :], brows("xt"))
        ag(quads, xtc, xtf)
        if STAGES >= 1:
            h_pass(xtf, 2, w1_sb, a1v_sb, hx1h, aux1d)
            ag(pairs, hx1h, hx1f)
            conv_pass(hx1f, hx1h, aux1d, bias1_sb, y0h)
        if STAGES >= 2:
            ag(pairs, y0h, y0f)
            lp_pass(y0f, y0h, y1h, None, None)
        if STAGES >= 3:
            ag(pairs, y1h, y1f)
            lp_pass(y1f, y0h, None, pool1, y1t)
        if STAGES >= 4:
            ag(quads, y1t, y1tf)
            h_pass(y1tf, HEADS, w2_sb, a2v_sb, hxh, aux2)
        if STAGES >= 5:
            ag(pairs, hxh, hxf)
            conv_pass(hxf, hxh, aux2, bias2_sb, y2h)
        if STAGES >= 6:
            ag(pairs, y2h, y2f)
            lp_pass(y2f, y2h, y3h, None, None)
            ag(pairs, y3h, y3f)
            lp_pass(y3f, y2h, None, pool2, None)
        nc.sync.dma_start(pool_out[0:NG, :], pool1[:])
        nc.sync.dma_start(pool_out[NG:2 * NG, :], pool2[:])
        if debug:
            for nm, t in (("y0h", y0h), ("y1h", y1h), ("y1t", y1t),
                          ("hxh", hxh), ("aux2", aux2), ("y2h", y2h),
                          ("y3h", y3h)):
                nc.sync.dma_start(dbg_t[nm][:], t[:])
    nc.compile()
    return nc


def _warm_devices():
    """Force PJRT client / NRT init + comm setup outside the timed call."""
    if _cached.get("devs_warm"):
        return
    try:
        import jax
        import jax.numpy as jnp
        devs = jax.devices()[:NCORES]
        outs = []
        for d in devs:
            x = jax.device_put(np.zeros(8, np.float32), d)
            outs.append(jnp.add(x, 1.0))
        jax.block_until_ready(outs)
    except Exception:
        pass
    _cached["devs_warm"] = True


def _run(nc, in_maps):
    from concourse.bass_utils import run_bass_kernel_spmd
    _warm_devices()
    t0 = time.time()
    res = run_bass_kernel_spmd(nc, in_maps, core_ids=list(range(NCORES)))
    dt = time.time() - t0
    _cached["device_wall_ns"] = (_cached.get("device_wall_ns", 0)
                                 + int(dt * 1e9))
    _cached.setdefault("call_walls", []).append(dt)
    _cached["last_result"] = res
    return res


# ------------------------------------------------------------------ host ---

def _lane16(a):
    return np.ascontiguousarray(a.reshape(-1, 16).T)


def _lane128(a):
    return np.ascontiguousarray(a.reshape(-1, 128).T).astype(BF)


def _split_halves(src, dst):
    out = []
    for e in (0, 1):
        m = dst >= HN if e else dst < HN
        ids = np.nonzero(m)[0]
        d = dst[ids]
        o = np.argsort(d)
        ids = ids[o]
        out.append((src[ids], d[o], ids))
    return out


def _pack_structure(halves, EB):
    T = HBLK * EB
    packed = []
    for e, (s, d, ids) in enumerate(halves):
        rel_all = d - e * HN
        blk = rel_all >> 7
        rel = rel_all & 127
        cnt = np.bincount(blk, minlength=HBLK)
        starts = np.concatenate([[0], np.cumsum(cnt)[:-1]])
        slot = np.arange(len(blk)) - starts[blk]
        tok = blk * EB + slot
        srcs = np.full(T, PAD_ROW, np.int64)
        srcs[tok] = s
        dstf = np.full(T, -1.0, np.float32)
        dstf[tok] = rel
        packed.append((tok, ids, _lane16(srcs.astype(np.int16)),
                       _lane128(dstf)))
    return packed


def _pool_x(x, bat, cnts):
    try:
        import scipy.sparse as sp
        S = sp.csr_matrix((np.ones(N, np.float32),
                           (bat, np.arange(N))), shape=(NG, N))
        return np.asarray(S @ x)
    except Exception:
        starts = np.searchsorted(bat, np.arange(NG))
        out = np.add.reduceat(x, starts, axis=0)
        return np.where((cnts > 0)[:, None], out, 0.0)


def _leaky(v):
    return np.where(v >= 0, v, NEG * v)


def kernel(x, edge_index, batch,
           conv1_W, conv1_asrc, conv1_adst, conv1_b,
           conv2_W, conv2_asrc, conv2_adst, conv2_b,
           mlp_W1, mlp_b1, mlp_W2, mlp_b2):
    _cached["device_wall_ns"] = 0
    _cached["call_walls"] = []
    t_host0 = time.time()
    x = np.asarray(x, np.float32)
    edge_index = np.asarray(edge_index)
    src = edge_index[0].astype(np.int64)
    dst = edge_index[1].astype(np.int64)
    bat = np.asarray(batch).astype(np.int64)
    W1 = np.asarray(conv1_W, np.float32)
    W2 = np.asarray(conv2_W, np.float32)
    a1s = np.asarray(conv1_asrc, np.float32)
    a1d = np.asarray(conv1_adst, np.float32)
    b1 = np.asarray(conv1_b, np.float32)
    b2 = np.asarray(conv2_b, np.float32)

    # ---- graph structure ----
    halves = _split_halves(src, dst)
    EBl = (int(max(np.bincount((d - e * HN) >> 7, minlength=HBLK).max()
                   for e, (_, d, _) in enumerate(halves))) + 127) // 128 * 128
    packed = _pack_structure(halves, EBl)
    TL = HBLK * EBl

    deg = np.bincount(dst, minlength=N).astype(np.float32)
    dis = np.where(deg > 0, 1.0 / np.sqrt(np.maximum(deg, 1.0)),
                   0.0).astype(np.float32)
    wlp = dis[src] * dis[dst] * ALPHA
    wl_lanes = []
    for e in (0, 1):
        tok, ids, _, _ = packed[e]
        flat = np.zeros(TL, np.float32)
        flat[tok] = wlp[ids]
        wl_lanes.append(_lane128(flat))

    cnts = np.bincount(bat, minlength=NG).astype(np.float32)
    batvs = []
    for e in (0, 1):
        nodes = e * HN + np.arange(HN)
        v = np.where(nodes < N, bat[np.minimum(nodes, N - 1)],
                     -1).astype(np.float32)
        batvs.append(_lane128(v))
    iota_arr = np.ascontiguousarray(np.broadcast_to(
        np.arange(128, dtype=np.float32), (128, 128))).astype(BF)
    ident_arr = np.eye(128, dtype=np.float32).astype(BF)
    bias1s = [np.ascontiguousarray(np.broadcast_to(
        b1[f * HID:(f + 1) * HID][None, :], (128, HID))).astype(np.float32)
        for f in range(HEADS)]
    bias2s = [np.ascontiguousarray(np.broadcast_to(
        b2[f * HID:(f + 1) * HID][None, :], (128, HID))).astype(np.float32)
        for f in range(HEADS)]
    w2s = [np.ascontiguousarray(W2[:, f * HID:(f + 1) * HID]).astype(BF)
           for f in range(HEADS)]
    a2vs = [np.ascontiguousarray(np.stack(
        [np.asarray(conv2_asrc, np.float32)[f],
         np.asarray(conv2_adst, np.float32)[f]], axis=1)).astype(BF)
        for f in range(HEADS)]

    # ---- ship x transposed, quarter-sharded ----
    xtb = np.zeros((IN_CH, NPAD), BF)
    xtb[:, :N] = np.ascontiguousarray(x.T).astype(BF)
    xts = [[np.ascontiguousarray(xtb[f * 64:(f + 1) * 64,
                                     e * HN:(e + 1) * HN])
            for e in (0, 1)] for f in range(HEADS)]
    w1s = [np.ascontiguousarray(W1[:, f * HID:(f + 1) * HID]).astype(BF)
           for f in range(HEADS)]
    a1vs = [np.ascontiguousarray(np.stack(
        [a1s[f], a1d[f]], axis=1)).astype(BF) for f in range(HEADS)]

    LAY = _blob_layout(EBl)

    tabs = []
    for e in (0, 1):
        t = np.zeros((LAY["tpad"], 128), BF)
        TR = LAY["trows"]
        t[0:TR] = packed[e][2].view(BF).reshape(TR, 128)
        t[TR:2 * TR] = packed[e][3].reshape(TR, 128)
        t[2 * TR:3 * TR] = wl_lanes[e].reshape(TR, 128)
        tabs.append(t)

    def pack_blob(f, e):
        parts = {
            "xt": xts[f][e],
            "tchunk": tabs[e][f * LAY["tch"]:(f + 1) * LAY["tch"]],
            "batv": batvs[e],
            "bias1": bias1s[f].view(BF),
            "bias2": bias2s[f].view(BF),
            "iota": iota_arr,
            "ident": ident_arr,
            "w1": w1s[f],
            "w2": w2s[f],
            "a1v": a1vs[f],
            "a2v": a2vs[f],
        }
        blob = np.empty((LAY["total"], 128), BF)
        for k, arr in parts.items():
            r0, nr = LAY[k]
            blob[r0:r0 + nr] = np.ascontiguousarray(arr).reshape(nr, 128)
        return blob

    key = EBl
    debug = bool(os.environ.get("K_DEBUG"))
    if _cached.get("key") != (key, debug):
        t0 = time.time()
        _cached["nc"] = _build_program(EBl, debug=debug)
        _cached["key"] = (key, debug)
        _cached["build_wall"] = time.time() - t0

    maps = []
    for c in range(NCORES):
        f, e = c // 2, c % 2
        maps.append({"blob": pack_blob(f, e)})
    _cached["host_pre_wall"] = time.time() - t_host0
    res = _run(_cached["nc"], maps)

    t_post0 = time.time()
    p1 = np.empty((NG, OUT1), np.float32)
    p2 = np.empty((NG, OUT1), np.float32)
    for c in range(NCORES):
        f, e = c // 2, c % 2
        po = np.asarray(res.results[c]["pool_out"])
        if e == 0:
            p1[:, f * HID:(f + 1) * HID] = po[0:NG]
            p2[:, f * HID:(f + 1) * HID] = po[NG:2 * NG]
        else:
            p1[:, f * HID:(f + 1) * HID] += po[0:NG]
            p2[:, f * HID:(f + 1) * HID] += po[NG:2 * NG]

    xp = _pool_x(x, bat, cnts)
    pooled = np.concatenate([xp, p1, p2], axis=1) / np.maximum(
        cnts, 1.0)[:, None]
    hdd = np.maximum(pooled @ np.asarray(mlp_W1, np.float32)
                     + np.asarray(mlp_b1, np.float32), 0.0)
    out = hdd @ np.asarray(mlp_W2, np.float32) + np.asarray(mlp_b2, np.float32)
    _cached["host_post_wall"] = time.time() - t_post0
    if debug:
        _cached["dbg"] = {"res": res, "packed": packed, "p1": p1, "p2": p2}
    return out.astype(np.float32)
